# revision 26
# baseline (speedup 1.0000x reference)
"""Chamfer distance loss on 8 Trainium2 NeuronCores.

Work-balanced sharding: only n-tiles containing valid (non-PAD) fg rows
contribute to the loss, so the kernel builds a schedule from x_lengths at
call time: every sample's valid n-tile range is cut into 4-tile *segments*
(512 rows each) and the segments are distributed evenly across the 8 cores
(SPMD: every core runs the same program over SEG_PER_CORE segments; dummy
segments are fed sentinel rows and ignored by the host).

Per tile (128 valid-ish fg rows x full 4096 prj):
  d2[n, m] = |fg_n|^2 + |prj_m|^2 - 2 fg_n . prj_m  is ONE K=16 bf16
  matmul per 512-wide PSUM bank over round-to-nearest bf16 hi/lo splits
  of the augmented operands (all hi/lo cross terms in the contraction ->
  ~f32 accuracy at full bf16 PE speed).  ScalarE drains each [128 x 2048]
  PSUM group into half of a [128 x 4096] SBUF bf16 tile with fused relu;
  VectorE folds the tile into a per-segment running colmin buffer
  (2x-mode tensor_tensor min) and reduces the row direction with a
  4-level 2x TT-min fold + small 1x reduce.

The per-core outputs (per-segment colmin partials [128, 4096] and
per-tile rowmin partials) are combined on the host: elementwise
min across segments of the same sample, min across the 128 tile rows,
relu, and the masked means.  PAD rows produce d2 ~ 3e8 and never win a
min; their rowmin contribution is dropped by the host mask.
"""

import sys

sys.path.insert(0, "/opt/trn_rl_repo")
sys.path.insert(0, "/root/.axon_site/_ro/trn_rl_repo")

import numpy as np

import concourse.bass as bass
import concourse.mybir as mybir
import concourse.tile as tile

B, N, M, D = 8, 4096, 4096, 3
PAD = 10000.0
P = 128  # partitions / rows per n-tile
MG = 2048  # m elements per PSUM group (4 banks)
N_GROUPS = M // MG  # 2
TILES_PER_SEG = 3  # n-tiles per schedule segment (384 rows)
BIG = 1.0e30

_cached = {}


def _patch_tile_commit_waits():
    """This walrus build rejects >1 sync-wait per instruction: hoist extra
    waits onto nofuse NOPs committed just before the instruction on the same
    engine (engine streams are in-order, so prefix waits are equivalent)."""
    if getattr(tile.TileContext, "_wait_split_patched", False):
        return
    orig_commit = tile.TileContext._commit_instruction

    def _commit_split(self, inst, lazy_reg_writes=True):
        si = getattr(inst, "sync_info", None)
        eng = getattr(inst, "engine", None)
        if (
            si is not None
            and si.on_wait
            and len(si.on_wait) > 1
            and eng is not None
            and eng != mybir.EngineType.Unassigned
        ):
            waits = list(si.on_wait)
            si.on_wait = waits[:1]
            for w in waits[1:]:
                nop = mybir.InstNoOp(
                    name=f"I-{self.nc.next_id()}",
                    sync_info=mybir.SyncInfo(on_wait=[w], on_update=[]),
                    bass_nofuse=True,
                    engine=eng,
                )
                orig_commit(self, nop, lazy_reg_writes=False)
        return orig_commit(self, inst, lazy_reg_writes)

    tile.TileContext._commit_instruction = _commit_split
    tile.TileContext._wait_split_patched = True


def _patch_tile_tail_drain():
    """This walrus build rejects >1 sync-wait on a TPB_CTRL (Drain)
    instruction; split the TileContext tail-drain's wait list across a chain
    of single-wait drains on the sync engine."""
    from bass_rust import ScopedClock

    def _drain_and_barrier(self, tick_clock, wait_clock):
        nc = self.nc
        drain_inst = nc.sync.drain()
        wait_clock.add_sem_waits(
            drain_inst.ins, ScopedClock({None: tick_clock.global_clock})
        )
        si = drain_inst.ins.sync_info
        waits = list(si.on_wait) if si is not None and si.on_wait else []
        if len(waits) > 1:
            si.on_wait = waits[:1]
            for w in waits[1:]:
                extra = nc.sync.drain()
                esi = extra.ins.sync_info
                if esi is None:
                    extra.ins.sync_info = type(si)(on_wait=[w], on_update=[])
                else:
                    esi.on_wait = [w]

        nc.all_engine_barrier()
        assert self.sems is not None
        popped = nc._tile_sem_poison_stack.pop()
        assert popped is self._sem_poison
        nc.clear_and_free_semaphores(list(self.sems.allocated().values()))
        nc.all_engine_barrier()

    tile.TileContext._drain_and_barrier = _drain_and_barrier


def _build_program(seg_per_core):
    _patch_tile_commit_waits()
    _patch_tile_tail_drain()
    f32 = mybir.dt.float32
    bf16 = mybir.dt.bfloat16
    Alu = mybir.AluOpType
    Ax = mybir.AxisListType

    slots = seg_per_core * TILES_PER_SEG  # n-tile slots per core
    nc = bass.Bass("TRN2", target_bir_lowering=False, debug=False, num_devices=B)
    fg_in = nc.dram_tensor("fg_c", [16, slots * P], bf16, kind="ExternalInput").ap()
    prj_in = nc.dram_tensor(
        "prj_c", [16, seg_per_core * M], bf16, kind="ExternalInput"
    ).ap()
    colmin_out = nc.dram_tensor(
        "colmin_out", [P, seg_per_core * M], bf16, kind="ExternalOutput"
    ).ap()
    rowmin_out = nc.dram_tensor(
        "rowmin_out", [P, slots], f32, kind="ExternalOutput"
    ).ap()

    with tile.TileContext(nc) as tc:
        with (
            tc.tile_pool(name="consts", bufs=1) as consts,
            tc.tile_pool(name="d2p", bufs=6) as d2p,
            tc.tile_pool(name="foldp", bufs=3) as foldp,
            tc.tile_pool(name="psum", bufs=2, space="PSUM") as psum,
        ):
            fg_sb = consts.tile([16, slots * P], bf16)
            prj_sb = consts.tile([16, seg_per_core * M], bf16)
            # per-segment chunks so segment 0's matmuls start early (the
            # 16-partition layout only drives 2 of 16 DMA ports)
            for k in range(seg_per_core):
                f0, f1 = k * TILES_PER_SEG * P, (k + 1) * TILES_PER_SEG * P
                nc.sync.dma_start(out=fg_sb[:, f0:f1], in_=fg_in[:, f0:f1])
                if k == 0:
                    nc.sync.dma_start(out=prj_sb[:, :512], in_=prj_in[:, :512])
                    nc.sync.dma_start(out=prj_sb[:, 512:MG], in_=prj_in[:, 512:MG])
                    nc.sync.dma_start(out=prj_sb[:, MG:M], in_=prj_in[:, MG:M])
                else:
                    nc.sync.dma_start(
                        out=prj_sb[:, k * M : (k + 1) * M],
                        in_=prj_in[:, k * M : (k + 1) * M],
                    )

            colmin = consts.tile([P, seg_per_core * M], bf16)
            rowmin_parts = consts.tile([P, slots], f32)
            # pre-load ScalarE's activation table while input DMAs stream
            warm = consts.tile([P, 1], f32)
            nc.vector.memset(warm[:], 0.0)
            nc.scalar.activation(warm[:], warm[:], mybir.ActivationFunctionType.Relu)

            for t in range(slots):
                seg = t // TILES_PER_SEG
                lhsT = fg_sb[:, t * P : (t + 1) * P]
                first = t % TILES_PER_SEG == 0
                c0 = seg * M
                cslice = colmin[:, c0 : c0 + M]
                if first:
                    # first tile of the segment: ScalarE drains straight
                    # into the colmin slice (no DVE copy needed)
                    d2 = cslice
                else:
                    d2t = d2p.tile([P, M], bf16)
                    d2 = d2t[:]
                for g in range(N_GROUPS):
                    grp = psum.tile([P, MG], f32, tag="grp")
                    for j in range(MG // 512):
                        m0 = seg * M + g * MG + j * 512
                        nc.tensor.matmul(
                            grp[:, j * 512 : (j + 1) * 512],
                            lhsT,
                            prj_sb[:, m0 : m0 + 512],
                            start=True,
                            stop=True,
                        )
                    # drain PSUM -> SBUF bf16 with fused relu on ScalarE
                    nc.scalar.activation(
                        d2[:, g * MG : (g + 1) * MG],
                        grp[:],
                        mybir.ActivationFunctionType.Relu,
                    )
                # whole-tile DVE ops (one op constant instead of two)
                if not first:
                    nc.vector.tensor_tensor(cslice, d2, cslice, Alu.min)
                if t == 0:
                    # pipeline fill: fold each half as soon as its drain lands
                    htmp = foldp.tile([P, MG], bf16, tag="h0")
                    racc = foldp.tile([P, 2], f32, tag="r0")
                    for g in range(N_GROUPS):
                        hh = htmp[:, g * (MG // 2) : (g + 1) * (MG // 2)]
                        dh = d2[:, g * MG : (g + 1) * MG]
                        nc.vector.tensor_tensor(
                            hh, dh[:, : MG // 2], dh[:, MG // 2 :], Alu.min
                        )
                        nc.vector.tensor_tensor(
                            hh[:, : MG // 4], hh[:, : MG // 4], hh[:, MG // 4 :], Alu.min
                        )
                        nc.vector.tensor_tensor(
                            hh[:, : MG // 8],
                            hh[:, : MG // 8],
                            hh[:, MG // 8 : MG // 4],
                            Alu.min,
                        )
                        nc.vector.tensor_reduce(
                            racc[:, g : g + 1], hh[:, : MG // 8], axis=Ax.X, op=Alu.min
                        )
                    nc.vector.tensor_reduce(
                        rowmin_parts[:, t : t + 1], racc[:], axis=Ax.X, op=Alu.min
                    )
                else:
                    # rowmin: four 2x-mode TT-min folds + small 1x reduce
                    h1 = foldp.tile([P, M // 2], bf16)
                    nc.vector.tensor_tensor(
                        h1[:], d2[:, : M // 2], d2[:, M // 2 :], Alu.min
                    )
                    nc.vector.tensor_tensor(
                        h1[:, : M // 4], h1[:, : M // 4], h1[:, M // 4 :], Alu.min
                    )
                    nc.vector.tensor_tensor(
                        h1[:, : M // 8], h1[:, : M // 8], h1[:, M // 8 : M // 4], Alu.min
                    )
                    nc.vector.tensor_tensor(
                        h1[:, : M // 16], h1[:, : M // 16], h1[:, M // 16 : M // 8], Alu.min
                    )
                    nc.vector.tensor_reduce(
                        rowmin_parts[:, t : t + 1],
                        h1[:, : M // 16],
                        axis=Ax.X,
                        op=Alu.min,
                    )
                # stream each finished segment's colmin back to DRAM
                if t % TILES_PER_SEG == TILES_PER_SEG - 1:
                    for gg in range(N_GROUPS):
                        o0 = seg * M + gg * MG
                        nc.sync.dma_start(
                            out=colmin_out[:, o0 : o0 + MG],
                            in_=colmin[:, o0 : o0 + MG],
                        )

            nc.sync.dma_start(out=rowmin_out, in_=rowmin_parts[:])

    return nc


def _split_bf16(x):
    """Round-to-nearest bf16 hi/lo split: x ~= hi + lo to ~16 mantissa bits."""
    import ml_dtypes

    hi = x.astype(np.float32).astype(ml_dtypes.bfloat16)
    lo = (x.astype(np.float32) - hi.astype(np.float32)).astype(ml_dtypes.bfloat16)
    return hi, lo


def _aug16(pts, sq):
    """[16, n] bf16 lhsT-side augmentation rows for d2 via one K=16 matmul:
    [a_hi, a_lo, 1, 1, -2f_hi(3), -2f_lo(3), -2f_hi(3), -2f_lo(3)]."""
    import ml_dtypes

    bf = ml_dtypes.bfloat16
    n = pts.shape[0]
    a_hi, a_lo = _split_bf16(sq)
    f_hi, f_lo = _split_bf16(pts)
    f2_hi = (-2.0 * f_hi.astype(np.float32)).astype(bf)
    f2_lo = (-2.0 * f_lo.astype(np.float32)).astype(bf)
    ones = np.ones(n, bf)
    return np.stack(
        [a_hi, a_lo, ones, ones]
        + [f2_hi[:, d] for d in range(3)]
        + [f2_lo[:, d] for d in range(3)]
        + [f2_hi[:, d] for d in range(3)]
        + [f2_lo[:, d] for d in range(3)]
    )


def _aug16_rhs(pts, sq):
    """[16, m] bf16 rhs-side augmentation rows:
    [1, 1, b_hi, b_lo, p_hi(3), p_hi(3), p_lo(3), p_lo(3)]."""
    import ml_dtypes

    bf = ml_dtypes.bfloat16
    m = pts.shape[0]
    b_hi, b_lo = _split_bf16(sq)
    p_hi, p_lo = _split_bf16(pts)
    ones = np.ones(m, bf)
    return np.stack(
        [ones, ones, b_hi, b_lo]
        + [p_hi[:, d] for d in range(3)]
        + [p_hi[:, d] for d in range(3)]
        + [p_lo[:, d] for d in range(3)]
        + [p_lo[:, d] for d in range(3)]
    )


def _build_schedule(lengths):
    """Split every sample's valid n-tile range into 4-tile segments and pack
    them into 8 equal per-core lists (padded with dummy segments)."""
    segs = []  # (sample, first_tile)
    for s in range(B):
        ntiles = int(lengths[s]) // P  # full tiles only; tail rows go to host
        for st in range(0, ntiles, TILES_PER_SEG):
            segs.append((s, st))
    seg_per_core = -(-len(segs) // B)
    while len(segs) < seg_per_core * B:
        segs.append(None)  # dummy
    cores = [segs[c * seg_per_core : (c + 1) * seg_per_core] for c in range(B)]
    return cores, seg_per_core


def _prep_inputs(fg, prj, lengths):
    import ml_dtypes

    bf = ml_dtypes.bfloat16
    cores, seg_per_core = _build_schedule(lengths)
    slots = seg_per_core * TILES_PER_SEG

    fg_f = fg.astype(np.float32)
    prj_f = prj.astype(np.float32)
    fg2 = (fg_f.astype(np.float64) ** 2).sum(-1).astype(np.float32)
    prj2 = (prj_f.astype(np.float64) ** 2).sum(-1).astype(np.float32)
    import ml_dtypes as _ml
    _pad = np.zeros((16, TILES_PER_SEG * P), _ml.bfloat16)
    _pad[0, :] = _ml.bfloat16(BIG)
    fg_aug = {
        s: np.concatenate([_aug16(fg_f[s], fg2[s]), _pad], axis=1) for s in range(B)
    }  # [16, N + pad]
    prj_aug = {s: _aug16_rhs(prj_f[s], prj2[s]) for s in range(B)}  # [16, M]
    pad_cols = np.full((16, P * TILES_PER_SEG), 0, bf)
    pad_cols[0, :] = bf(BIG)  # d2 of dummy rows = BIG + prj2 - 0 >> any real d2

    in_maps = []
    for c in range(B):
        fg_c = np.empty((16, slots * P), bf)
        prj_c = np.empty((16, seg_per_core * M), bf)
        for k, seg in enumerate(cores[c]):
            lo = k * TILES_PER_SEG * P
            hi = lo + TILES_PER_SEG * P
            if seg is None:
                fg_c[:, lo:hi] = pad_cols
                prj_c[:, k * M : (k + 1) * M] = prj_aug[0]
            else:
                s, st = seg
                r0 = st * P
                fg_c[:, lo:hi] = fg_aug[s][:, r0 : r0 + TILES_PER_SEG * P]
                prj_c[:, k * M : (k + 1) * M] = prj_aug[s]
        in_maps.append(
            {"fg_c": np.ascontiguousarray(fg_c), "prj_c": np.ascontiguousarray(prj_c)}
        )
    return in_maps, cores, seg_per_core


def _combine(results, cores, seg_per_core, lengths, _tail_fg, _tail_prj):
    """Host-side reduction of the per-core partials to the scalar loss."""
    colmin = {}  # sample -> running [P, M] f32 min
    rowsum = np.zeros(B, np.float64)  # per-sample masked sum of rowmins
    for c in range(B):
        cm = np.asarray(results[c]["colmin_out"], dtype=np.float32)
        rm = np.asarray(results[c]["rowmin_out"], dtype=np.float32)
        cm = cm.reshape(P, seg_per_core, M)
        rm = rm.reshape(P, seg_per_core * TILES_PER_SEG)
        for k, seg in enumerate(cores[c]):
            if seg is None:
                continue
            s, st = seg
            prev = colmin.get(s)
            cur = cm[:, k, :]
            colmin[s] = cur if prev is None else np.minimum(prev, cur)
            t0 = (int(lengths[s]) // P) * P
            for tt in range(TILES_PER_SEG):
                n0 = (st + tt) * P
                # only full floor-tiles live on the device; the partial
                # tail (and anything beyond) is computed on the host
                if n0 + P > t0:
                    continue
                rmin = rm[:, k * TILES_PER_SEG + tt]
                rowsum[s] += np.maximum(rmin, 0.0).sum()
    total = 0.0
    for s in range(B):
        L = int(lengths[s])
        # stripped tail rows (L//P*P .. L) computed directly on the host
        t0 = (L // P) * P
        cmin = colmin.get(s)
        cmin_cols = None if cmin is None else np.maximum(cmin, 0.0).min(axis=0)
        if L > t0:
            ftail = _tail_fg[s][t0:L].astype(np.float64)
            d2t = (
                (ftail[:, None, :] - _tail_prj[s][None, :, :].astype(np.float64)) ** 2
            ).sum(-1)
            d2t = np.maximum(d2t, 0.0)
            rowsum[s] += d2t.min(axis=1).sum()
            tmin = d2t.min(axis=0)
            cmin_cols = tmin if cmin_cols is None else np.minimum(cmin_cols, tmin)
        cham_x = rowsum[s] / L
        cham_y = cmin_cols.mean()
        total += cham_x + cham_y
    return np.float32(total / B)


def _run(in_maps, seg_per_core, trace=False):
    from concourse.bass_utils import run_bass_kernel_spmd

    key = ("nc", seg_per_core)
    if key not in _cached:
        _cached[key] = _build_program(seg_per_core)
    return run_bass_kernel_spmd(_cached[key], in_maps, list(range(B)), trace=trace)


def kernel(fg_points, prj_points, x_lengths, _trace=False):
    fg = np.asarray(fg_points)
    prj = np.asarray(prj_points)
    lengths = np.asarray(x_lengths)
    in_maps, cores, seg_per_core = _prep_inputs(fg, prj, lengths)
    res = _run(in_maps, seg_per_core, trace=_trace)
    out = _combine(res.results, cores, seg_per_core, lengths, fg, prj)
    if _trace:
        return out, res
    return out
